# revision 26
# baseline (speedup 1.0000x reference)
"""GCN (3-layer + MLP head) on 8 Trainium2 NeuronCores.

Strategy (graph-parallel, per sharding hint):
  - Nodes sharded 8 ways by id; each core owns its dst-node shard plus the
    edges (excl. self-loops) pointing into it.  Within each shard, nodes are
    permuted into blocks by a greedy 4-vector bin-packing so per-(quarter,
    block) in-edge counts are flat across cores (the shared SPMD schedule
    pays max-over-cores per cell).
  - Phase A (per core): Hs = dinv * (X_shard @ W1) as a bf16 row table kept
    in SBUF + written to DRAM.  The table AllGather is split into 4 block-
    aligned quarter pieces so the first fires as soon as the first 24 blocks
    are done and message passing starts ~110us in.  The per-block
    accumulator is initialized right here to dinv*(XW1) + sqrt(deg) (x) b1
    == self-loop message + pre-scaled bias (so self-loops are never
    gathered and no per-cell bias matmuls exist).
  - MP phase: edges sorted by (src-quarter, dst block) and packed densely
    at cnt_max granularity; dma_gather pulls 256B bf16 rows in 4096-slot
    chunks (short final chunk per quarter); one batched DVE is_equal with
    0-stride broadcast APs builds all one-hot S columns of a chunk; per
    (group x block) incidence a PE matmul scatter-sums messages into the
    dst-block PSUM, accumulated into a [128, nb, F] f32 SBUF accumulator.
  - GCN layers 2+3 collapse to weighted node sums (host norm vectors);
    p = sum_d u2[d]*relu(h1[d]) is a PE reduction, combined across cores by
    a tiny AllGather + ones-vector matmul (the 512B mesh AllReduce costs
    164us; this path ~25us).  Tail MLP replicated; core 0's output is
    returned.
"""
import math
import numpy as np
import ml_dtypes

import concourse.bass as bass
import concourse.tile as tile
from concourse import bacc, mybir
from concourse.bass_utils import run_bass_kernel_spmd

N_CORES = 8
F = 128          # feature dim (all layers)
BLK = 128        # dst-block size (PSUM partition dim)
GRP = 128        # edges per matmul group (PE contraction dim)
CHUNK = 8192     # slots per dma_gather call
CGRPS = CHUNK // GRP
NQUEUES = 4
SENT = 130.0     # dst offset sentinel for padded edge slots (!= 0..127)

BF16 = ml_dtypes.bfloat16


# ----------------------------------------------------------------------------
# host preprocessing: shard, normalize, sort, pack, schedule
# ----------------------------------------------------------------------------
def _preprocess(graph, edge_index, rates, params):
    N = graph.shape[0]
    assert N % N_CORES == 0
    shard = N // N_CORES                      # real nodes per core
    nb = math.ceil(shard / BLK)               # dst blocks per core
    shard_pad = nb * BLK
    nq = 4                                    # src quarters (int16 idx range)

    src = np.asarray(edge_index[0], np.int64)
    dst = np.asarray(edge_index[1], np.int64)

    # degrees / normalization (float64 host precompute of scalar edge data)
    deg = np.bincount(dst, minlength=N).astype(np.float64) + 1.0
    dinv = deg ** -0.5
    sq = deg ** 0.5
    u1 = dinv * (np.bincount(src, weights=dinv[dst], minlength=N) + dinv)
    y = u1 * dinv
    u2 = dinv * (np.bincount(src, weights=y[dst], minlength=N) + y)
    S1 = float(u1.sum())
    S2 = float(u2.sum())

    # quarters = block-aligned slices of each shard (mod-shard interleave)
    # so that AllGather_q covers every core's q-th block range and can be
    # issued as soon as those phase-A blocks are done.
    qnb = [24, 25, 24, 25]                    # blocks per quarter (sum = nb)
    assert sum(qnb) == nb
    qb0 = np.cumsum([0] + qnb)                # block offsets [nq+1]
    qsz = [q * BLK for q in qnb]              # rows per (core, quarter)
    assert max(qsz) * N_CORES - 1 <= np.iinfo(np.int16).max
    blk2q = np.zeros(nb, np.int64)
    for q in range(nq):
        blk2q[qb0[q]:qb0[q + 1]] = q

    # ---- balance per-(quarter, block) in-edge counts by permuting each
    #      shard's node->block assignment (greedy 4-vector bin packing,
    #      2 rounds since src quarters depend on the placement).  The
    #      shared schedule pays max-over-cores per cell, so flattening the
    #      spread cuts gather slots on both the Q7 and SDMA side.
    lpos = np.arange(N) % shard               # local position within shard
    cap = np.full(nb, BLK, np.int64)
    cap[-1] = shard - (nb - 1) * BLK
    for _ in range(3):
        src_q = blk2q[lpos[src] // BLK]
        qdeg = np.zeros((N, nq), np.int64)
        np.add.at(qdeg, (dst, src_q), 1)
        new_lpos = np.empty(N, np.int64)
        for c in range(N_CORES):
            sl = slice(c * shard, (c + 1) * shard)
            qd = qdeg[sl].astype(np.float64)
            order = np.argsort(-qd.sum(1), kind="stable")
            loads = np.zeros((nb, nq))
            fill = np.zeros(nb, np.int64)
            pos = np.empty(shard, np.int64)
            for i in order:
                sc = (loads + qd[i]).max(1)
                sc[fill >= cap] = np.inf
                b = int(np.argmin(sc))
                loads[b] += qd[i]
                pos[i] = b * BLK + fill[b]
                fill[b] += 1
            new_lpos[sl] = pos
        lpos = new_lpos

    # per-core edge attribution (NO self loops in the gather)
    core_of = dst // shard
    l_src = lpos[src]                         # local row of src in its shard
    sblk = l_src // BLK
    quarter = blk2q[sblk]
    c_src = src // shard
    qsz_a = np.array(qsz, np.int64)
    qoff_a = qb0[:-1] * BLK
    loc_idx = c_src * qsz_a[quarter] + (l_src - qoff_a[quarter])
    dl = lpos[dst]
    blk_of = dl // BLK
    off_of = (dl % BLK).astype(np.int64)

    # counts per (core, quarter, block); shared schedule uses max over cores
    cnt = np.zeros((N_CORES, nq, nb), np.int64)
    np.add.at(cnt, (core_of, quarter, blk_of), 1)
    cnt_max = cnt.max(axis=0)                       # [nq, nb]

    # dense cell offsets within each quarter's slot space
    cell_off = np.zeros((nq, nb + 1), np.int64)
    for q in range(nq):
        cell_off[q, 1:] = np.cumsum(cnt_max[q])
    SQ = cell_off[:, -1]                            # slots per quarter

    # shared chunk-level schedule
    sched_chunks = []   # dicts: q, k, nreg, entries=[(j, gl, b, first, last)]
    chunk_index = {}    # (q, k) -> global chunk idx
    for q in range(nq):
        nck = math.ceil(int(SQ[q]) / CHUNK)
        per_chunk = [[] for _ in range(nck)]
        for b in range(nb):
            o, e = int(cell_off[q, b]), int(cell_off[q, b + 1])
            g0, g1 = o // GRP, (e - 1) // GRP
            for g in range(g0, g1 + 1):
                ch = g // CGRPS
                per_chunk[ch].append((g - ch * CGRPS, b, g == g0, g == g1))
        for k in range(nck):
            entries = [(j, gl, b, fi, la)
                       for j, (gl, b, fi, la) in enumerate(per_chunk[k])]
            nreg = min(CHUNK, int(SQ[q]) - k * CHUNK)
            chunk_index[(q, k)] = len(sched_chunks)
            sched_chunks.append(dict(q=q, k=k, nreg=nreg, entries=entries))
    nchunk = len(sched_chunks)
    ncol = max(len(c["entries"]) for c in sched_chunks)

    # per-core edge data arrays in schedule order
    idx16 = np.zeros((N_CORES, nchunk, 128, CHUNK // 16), np.int16)
    dstid = np.full((N_CORES, nchunk, 128, ncol), SENT, BF16)
    e_ar = np.arange(CHUNK)
    for c in range(N_CORES):
        m = core_of == c
        q_c, b_c = quarter[m], blk_of[m]
        il_c, off_c = loc_idx[m], off_of[m]
        order = np.lexsort((b_c, q_c))
        q_c, b_c, il_c, off_c = q_c[order], b_c[order], il_c[order], off_c[order]
        key = q_c * nb + b_c
        for q in range(nq):
            nck = math.ceil(int(SQ[q]) / CHUNK)
            SQp = nck * CHUNK
            iv = np.zeros(SQp, np.int64)
            ov = np.full(SQp, SENT)
            cell = np.full(SQp, -1, np.int64)
            for b in range(nb):
                cc = int(cnt[c, q, b])
                o = int(cell_off[q, b])
                s0 = np.searchsorted(key, q * nb + b, 'left')
                iv[o:o + cc] = il_c[s0:s0 + cc]
                ov[o:o + cc] = off_c[s0:s0 + cc]
                cell[o:o + cc] = b
            for k in range(nck):
                ci = chunk_index[(q, k)]
                sl = slice(k * CHUNK, (k + 1) * CHUNK)
                tmp = np.zeros((16, CHUNK // 16), np.int16)
                tmp[e_ar % 16, e_ar // 16] = iv[sl]
                idx16[c, ci] = np.tile(tmp, (8, 1))
                for (j, gl, b, fi, la) in sched_chunks[ci]["entries"]:
                    s = k * CHUNK + gl * GRP
                    seg = slice(s, s + GRP)
                    dstid[c, ci][:, j] = np.where(
                        cell[seg] == b, ov[seg], SENT).astype(BF16)

    # phase A inputs
    X = np.asarray(graph, np.float32)
    xt = np.zeros((N_CORES, F, shard_pad), BF16)
    dinv_pm = np.zeros((N_CORES, BLK, nb), np.float32)
    sq_pm = np.zeros((N_CORES, BLK, nb), np.float32)
    u2_pm = np.zeros((N_CORES, BLK, nb), np.float32)
    for c in range(N_CORES):
        lp = lpos[c * shard: (c + 1) * shard]
        p2n = np.empty(shard, np.int64)
        p2n[lp] = np.arange(shard)            # position -> local node idx
        xs = X[c * shard: (c + 1) * shard][p2n]
        xt[c, :, :shard] = xs.T.astype(BF16)
        dv = np.zeros(shard_pad); dv[:shard] = dinv[c * shard:(c + 1) * shard][p2n]
        sv = np.zeros(shard_pad); sv[:shard] = sq[c * shard:(c + 1) * shard][p2n]
        uv = np.zeros(shard_pad); uv[:shard] = u2[c * shard:(c + 1) * shard][p2n]
        dinv_pm[c] = dv.reshape(nb, BLK).T
        sq_pm[c] = sv.reshape(nb, BLK).T
        u2_pm[c] = uv.reshape(nb, BLK).T

    p = params
    col = lambda v: np.asarray(v, np.float32).reshape(-1, 1)
    iota_bf = np.tile(np.arange(BLK, dtype=np.float32)[None, :],
                      (128, 1)).astype(BF16)
    common = dict(
        w1=np.asarray(p["conv1_W"], np.float32).astype(BF16),
        b1t=np.tile(np.asarray(p["conv1_b"], np.float32).reshape(1, F),
                    (BLK, 1)),
        iotabf=iota_bf,
        rates_col=col(rates),
        encw1=np.asarray(p["enc_W1"], np.float32),
        encb1=col(p["enc_b1"]),
        encw2=np.asarray(p["enc_W2"], np.float32),
        encb2=col(p["enc_b2"]),
        w2a=np.asarray(p["conv2_W"], np.float32)[:F],
        w2b=np.asarray(p["conv2_W"], np.float32)[F:],
        b2col=col(p["conv2_b"]),
        s1col=np.full((F, 1), S1, np.float32),
        s2col=np.full((F, 1), S2, np.float32),
        w3=np.asarray(p["conv3_W"], np.float32),
        b3col=col(p["conv3_b"]),
        hidw=np.asarray(p["hid_W"], np.float32),
        hidb=np.asarray(p["hid_b"], np.float32).reshape(2, F).T,
        hid2wa=np.asarray(p["hid2_W"], np.float32)[:F],
        hid2wb=np.asarray(p["hid2_W"], np.float32)[F:],
        hid2b=col(p["hid2_b"]),
        finw=np.asarray(p["fin_W"], np.float32),
        finb=col(p["fin_b"]),
    )
    in_maps = []
    for c in range(N_CORES):
        m = dict(common)
        m.update(
            xt=xt[c], dinv=dinv_pm[c], sqdeg=sq_pm[c], u2c=u2_pm[c],
            idx16=idx16[c], dstid=dstid[c],
        )
        in_maps.append(m)

    dims = dict(N=N, shard=shard, shard_pad=shard_pad, nb=nb, nq=nq,
                qnb=tuple(qnb), nchunk=nchunk, ncol=ncol)
    return in_maps, sched_chunks, dims


# ----------------------------------------------------------------------------
# device program
# ----------------------------------------------------------------------------
def _build(sched_chunks, dims):
    nb, nq, nchunk = dims["nb"], dims["nq"], dims["nchunk"]
    ncol = dims["ncol"]
    shard_pad, N = dims["shard_pad"], dims["N"]
    qnb = list(dims["qnb"])
    qb0 = [0]
    for q in qnb:
        qb0.append(qb0[-1] + q)
    f32, bf16, i16 = mybir.dt.float32, mybir.dt.bfloat16, mybir.dt.int16

    nc = bacc.Bacc("TRN2", target_bir_lowering=False, debug=False,
                   num_devices=N_CORES, num_swdge_queues=NQUEUES)
    I = lambda name, shape, dt=f32: nc.dram_tensor(name, shape, dt, kind="ExternalInput")
    xt_e = I("xt", [F, shard_pad], bf16)
    w1_e = I("w1", [F, F], bf16)
    b1t_e = I("b1t", [BLK, F])
    dinv_e = I("dinv", [BLK, nb]); sq_e = I("sqdeg", [BLK, nb])
    u2_e = I("u2c", [BLK, nb])
    idx_e = I("idx16", [nchunk, 128, CHUNK // 16], i16)
    dst_e = I("dstid", [nchunk, 128, ncol], bf16)
    iota_e = I("iotabf", [128, BLK], bf16)
    rates_e = I("rates_col", [16, 1])
    encw1_e = I("encw1", [16, 8]); encb1_e = I("encb1", [8, 1])
    encw2_e = I("encw2", [8, F]); encb2_e = I("encb2", [F, 1])
    w2a_e = I("w2a", [F, F]); w2b_e = I("w2b", [F, F]); b2_e = I("b2col", [F, 1])
    s1_e = I("s1col", [F, 1]); s2_e = I("s2col", [F, 1])
    w3_e = I("w3", [F, F]); b3_e = I("b3col", [F, 1])
    hidw_e = I("hidw", [F, 2 * F]); hidb_e = I("hidb", [F, 2])
    hid2wa_e = I("hid2wa", [F, F]); hid2wb_e = I("hid2wb", [F, F])
    hid2b_e = I("hid2b", [F, 1])
    finw_e = I("finw", [F, 2]); finb_e = I("finb", [2, 1])
    out_e = nc.dram_tensor("out", [2, 1], f32, kind="ExternalOutput")

    hs_shard_q = [nc.dram_tensor(f"hs_shard{q}", [qnb[q] * BLK, F], bf16)
                  for q in range(nq)]
    hs_full_q = [nc.dram_tensor(f"hs_full{q}", [N_CORES * qnb[q] * BLK, F],
                                bf16, addr_space="Shared")
                 for q in range(nq)]
    p_dram = nc.dram_tensor("p_dram", [1, F], f32)
    p_all = nc.dram_tensor("p_all", [N_CORES, F], f32, addr_space="Shared")
    groups_all = list(range(N_CORES))

    with tile.TileContext(nc) as tc:
        with (
            tc.tile_pool(name="const", bufs=1) as cpool,
            tc.tile_pool(name="xa", bufs=3) as xapool,
            tc.tile_pool(name="work", bufs=6) as wpool,
            tc.tile_pool(name="gat", bufs=3) as gpool,
            tc.tile_pool(name="sstile", bufs=2) as spool,
            tc.tile_pool(name="accum", bufs=1) as apool,
            tc.tile_pool(name="ps", bufs=4, space="PSUM") as pspool,
            tc.tile_pool(name="psp", bufs=1, space="PSUM") as psppool,
        ):
            # ---- constants / small inputs
            w1_sb = cpool.tile([F, F], bf16); nc.sync.dma_start(w1_sb[:], w1_e[:])
            b1t_sb = cpool.tile([BLK, F], f32); nc.sync.dma_start(b1t_sb[:], b1t_e[:])
            dinv_sb = cpool.tile([BLK, nb], f32); nc.sync.dma_start(dinv_sb[:], dinv_e[:])
            sq_sb = cpool.tile([BLK, nb], f32); nc.sync.dma_start(sq_sb[:], sq_e[:])
            u2_sb = cpool.tile([BLK, nb], f32); nc.sync.dma_start(u2_sb[:], u2_e[:])
            iota_sb = cpool.tile([128, BLK], bf16); nc.sync.dma_start(iota_sb[:], iota_e[:])

            hs_big = cpool.tile([BLK, nb, F], bf16)
            accum = apool.tile([BLK, nb, F], f32)

            # ---- phase A: table Hs = dinv*(X@W1) + accum init
            #      accum[:,n,:] = dinv*(XW1) + sqrt(deg) (x) b1
            #      (= self-loop message + pre-scaled bias)
            #      xt loads batched 4 blocks; hs stores batched per quarter
            #      chunklet; AllGather_q fires as soon as quarter q is done.
            for n4 in range(0, nb, 4):
                hi = min(n4 + 4, nb)
                w = hi - n4
                xts = xapool.tile([F, 4 * BLK], bf16, tag="xts")
                nc.sync.dma_start(xts[:, :w * BLK],
                                  xt_e[:, n4 * BLK: hi * BLK])
                for n in range(n4, hi):
                    s = (n - n4) * BLK
                    psA = pspool.tile([BLK, F], f32, tag="ps")
                    nc.tensor.matmul(psA[:], xts[:, s: s + BLK], w1_sb[:],
                                     start=True, stop=True)
                    nc.scalar.activation(
                        hs_big[:, n, :], psA[:],
                        mybir.ActivationFunctionType.Copy,
                        scale=dinv_sb[:, n: n + 1],
                    )
                    nc.vector.tensor_copy(accum[:, n, :], hs_big[:, n, :])
                    nc.vector.scalar_tensor_tensor(
                        out=accum[:, n, :], in0=b1t_sb[:],
                        scalar=sq_sb[:, n: n + 1], in1=accum[:, n, :],
                        op0=mybir.AluOpType.mult, op1=mybir.AluOpType.add,
                    )
            # quarter-sliced table stores + per-quarter AllGathers
            for q in range(nq):
                for b0 in range(qb0[q], qb0[q + 1], 4):
                    b1_ = min(b0 + 4, qb0[q + 1])
                    r0 = (b0 - qb0[q]) * BLK
                    dst_ap = hs_shard_q[q][r0: r0 + (b1_ - b0) * BLK, :]
                    dst_ap = dst_ap.rearrange("(j p) f -> p j f", p=BLK)
                    nc.sync.dma_start(dst_ap, hs_big[:, b0: b1_, :])
                nc.gpsimd.collective_compute(
                    "AllGather", mybir.AluOpType.bypass,
                    replica_groups=[groups_all],
                    ins=[hs_shard_q[q][:]], outs=[hs_full_q[q][:]],
                )

            # ---- message passing over chunks
            ps_p = psppool.tile([1, F], f32)
            p_started = False
            psum_of_block = {}
            for ci, chd in enumerate(sched_chunks):
                q = chd["q"]
                # short final chunks: gather only the slots the schedule uses
                ng = min(CGRPS, -(-chd["nreg"] // GRP))
                ni = ng * GRP
                idxt = wpool.tile([128, CHUNK // 16], i16, tag="idxt")
                nc.sync.dma_start(idxt[:, : ni // 16], idx_e[ci][:, : ni // 16])
                dstt = wpool.tile([128, ncol], bf16, tag="dstt")
                nc.sync.dma_start(dstt[:], dst_e[ci])
                G = gpool.tile([128, CGRPS, F], bf16, tag="G")
                nc.gpsimd.dma_gather(
                    out_ap=G[:, :ng, :],
                    in_ap=hs_full_q[q][:],
                    idxs_ap=idxt[:, : ni // 16],
                    num_idxs=ni, num_idxs_reg=ni, elem_size=F,
                    single_packet=False, queue_num=(ci % NQUEUES),
                )
                S_all = spool.tile([128, ncol, BLK], bf16, tag="S")
                nc.vector.tensor_tensor(
                    out=S_all[:],
                    in0=dstt[:].unsqueeze(2).to_broadcast([128, ncol, BLK]),
                    in1=iota_sb[:].unsqueeze(1).to_broadcast([128, ncol, BLK]),
                    op=mybir.AluOpType.is_equal,
                )
                for (j, gl, b, first, last) in chd["entries"]:
                    if first:
                        psb = pspool.tile([BLK, F], f32, tag="ps")
                        psum_of_block[b] = psb
                    psb = psum_of_block[b]
                    nc.tensor.matmul(
                        psb[:], S_all[:, j, :], G[:, gl, :],
                        start=first, stop=last,
                    )
                    if last:
                        del psum_of_block[b]
                        if q < nq - 1:
                            nc.vector.tensor_add(accum[:, b, :], accum[:, b, :], psb[:])
                        else:
                            nc.vector.tensor_add(accum[:, b, :], accum[:, b, :], psb[:])
                            h1b = spool.tile([BLK, F], f32, tag="h1b")
                            nc.scalar.activation(
                                h1b[:], accum[:, b, :],
                                mybir.ActivationFunctionType.Relu,
                                scale=dinv_sb[:, b: b + 1],
                            )
                            nc.tensor.matmul(
                                ps_p[:], u2_sb[:, b: b + 1], h1b[:],
                                start=not p_started, stop=(b == nb - 1),
                                skip_group_check=True,
                            )
                            p_started = True

            # ---- combine p across cores: tiny AllGather + ones reduction
            p_sb = cpool.tile([1, F], f32)
            nc.vector.tensor_copy(p_sb[:], ps_p[:])
            nc.sync.dma_start(p_dram[:], p_sb[:])
            nc.gpsimd.collective_compute(
                "AllGather", mybir.AluOpType.bypass,
                replica_groups=[groups_all],
                ins=[p_dram[:]], outs=[p_all[:]],
            )
            p_all_sb = cpool.tile([N_CORES, F], f32)
            nc.sync.dma_start(p_all_sb[:], p_all[:])
            ones8 = cpool.tile([N_CORES, 1], f32)
            nc.vector.memset(ones8[:], 1.0)
            ps_pr = pspool.tile([1, F], f32, tag="ps")
            nc.tensor.matmul(ps_pr[:], ones8[:], p_all_sb[:], start=True, stop=True)
            p_row = cpool.tile([1, F], f32)
            nc.vector.tensor_copy(p_row[:], ps_pr[:])
            id1 = cpool.tile([1, 1], f32)
            nc.vector.memset(id1[:], 1.0)
            psT = pspool.tile([F, 1], f32, tag="ps")
            nc.tensor.transpose(psT[:], p_row[:], id1[:])
            p_col = cpool.tile([F, 1], f32)
            nc.vector.tensor_copy(p_col[:], psT[:])

            # ---- replicated tail MLP (column-vector chain on PE/ACT/DVE)
            tl = cpool
            def ld(e, shape, dt=f32):
                t = tl.tile(shape, dt, tag=f"c_{e.name}")
                nc.sync.dma_start(t[:], e[:])
                return t
            rates_sb = ld(rates_e, [16, 1]); encw1_sb = ld(encw1_e, [16, 8])
            encb1_sb = ld(encb1_e, [8, 1]); encw2_sb = ld(encw2_e, [8, F])
            encb2_sb = ld(encb2_e, [F, 1])
            w2a_sb = ld(w2a_e, [F, F]); w2b_sb = ld(w2b_e, [F, F])
            b2_sb = ld(b2_e, [F, 1]); s1_sb = ld(s1_e, [F, 1]); s2_sb = ld(s2_e, [F, 1])
            w3_sb = ld(w3_e, [F, F]); b3_sb = ld(b3_e, [F, 1])
            hidw_sb = ld(hidw_e, [F, 2 * F]); hidb_sb = ld(hidb_e, [F, 2])
            hid2wa_sb = ld(hid2wa_e, [F, F]); hid2wb_sb = ld(hid2wb_e, [F, F])
            hid2b_sb = ld(hid2b_e, [F, 1])
            finw_sb = ld(finw_e, [F, 2]); finb_sb = ld(finb_e, [2, 1])

            pst = pspool.tile([F, 2], f32, tag="ps")  # scratch psum, 2 cols

            # r1 = relu(encW1^T rates + encb1)   [8,1]
            nc.tensor.matmul(pst[:8, 0:1], encw1_sb[:], rates_sb[:], start=True, stop=True)
            r1 = tl.tile([8, 1], f32)
            nc.scalar.activation(r1[:], pst[:8, 0:1],
                                 mybir.ActivationFunctionType.Relu, bias=encb1_sb[:])
            # r2 = encW2^T r1 + encb2            [F,1]
            nc.tensor.matmul(pst[:, 1:2], encw2_sb[:], r1[:], start=True, stop=True)
            r2 = tl.tile([F, 1], f32)
            nc.vector.tensor_add(r2[:], pst[:, 1:2], encb2_sb[:])
            # m_r = S2 * r2
            mr = tl.tile([F, 1], f32)
            nc.vector.tensor_mul(mr[:], r2[:], s2_sb[:])
            # u1h2 = W2a^T p + W2b^T m_r + S1*b2 ; q = u1h2 / N
            pst2 = pspool.tile([F, 1], f32, tag="ps")
            nc.tensor.matmul(pst2[:], w2a_sb[:], p_col[:], start=True, stop=False)
            nc.tensor.matmul(pst2[:], w2b_sb[:], mr[:], start=False, stop=True)
            sb2 = tl.tile([F, 1], f32)
            nc.vector.tensor_mul(sb2[:], b2_sb[:], s1_sb[:])
            qv = tl.tile([F, 1], f32)
            nc.vector.tensor_add(qv[:], pst2[:], sb2[:])
            nc.vector.tensor_scalar_mul(qv[:], qv[:], 1.0 / dims["N"])
            # m3 = W3^T q + b3
            pst3 = pspool.tile([F, 1], f32, tag="ps")
            nc.tensor.matmul(pst3[:], w3_sb[:], qv[:], start=True, stop=True)
            m3 = tl.tile([F, 1], f32)
            nc.vector.tensor_add(m3[:], pst3[:], b3_sb[:])
            # g1 = relu(hidW^T m3 + hidb)  [256] as two cols
            g1a = tl.tile([F, 1], f32); g1b = tl.tile([F, 1], f32)
            nc.tensor.matmul(pst[:, 0:1], hidw_sb[:, :F], m3[:], start=True, stop=True)
            nc.scalar.activation(g1a[:], pst[:, 0:1],
                                 mybir.ActivationFunctionType.Relu, bias=hidb_sb[:, 0:1])
            nc.tensor.matmul(pst[:, 1:2], hidw_sb[:, F:], m3[:], start=True, stop=True)
            nc.scalar.activation(g1b[:], pst[:, 1:2],
                                 mybir.ActivationFunctionType.Relu, bias=hidb_sb[:, 1:2])
            # g2 = relu(hid2W^T g1 + hid2b)  [F,1]
            pst4 = pspool.tile([F, 1], f32, tag="ps")
            nc.tensor.matmul(pst4[:], hid2wa_sb[:], g1a[:], start=True, stop=False)
            nc.tensor.matmul(pst4[:], hid2wb_sb[:], g1b[:], start=False, stop=True)
            g2 = tl.tile([F, 1], f32)
            nc.scalar.activation(g2[:], pst4[:],
                                 mybir.ActivationFunctionType.Relu, bias=hid2b_sb[:])
            # out = finW^T g2 + finb  [2,1]
            pst5 = pspool.tile([2, 1], f32, tag="ps")
            nc.tensor.matmul(pst5[:], finw_sb[:], g2[:], start=True, stop=True)
            outv = tl.tile([2, 1], f32)
            nc.vector.tensor_add(outv[:], pst5[:], finb_sb[:])
            nc.sync.dma_start(out_e[:], outv[:])

    nc.compile()
    return nc


_CACHE = {}
LAST_RESULTS = None


def kernel(**inputs):
    graph = np.asarray(inputs["graph"], np.float32)
    edge_index = np.asarray(inputs["edge_index"], np.int64)
    rates = np.asarray(inputs["rates"], np.float32)
    params = {k: np.asarray(v) for k, v in inputs.items()
              if k not in ("graph", "edge_index", "rates")}
    in_maps, sched, dims = _preprocess(graph, edge_index, rates, params)
    key = (dims["nchunk"], dims["ncol"], dims["shard_pad"],
           tuple((c["q"], c["nreg"], tuple(c["entries"])) for c in sched))
    if key not in _CACHE:
        _CACHE[key] = _build(sched, dims)
    nc = _CACHE[key]
    import os
    trace = bool(int(os.environ.get("GCN_TRACE", "0")))
    res = run_bass_kernel_spmd(nc, in_maps, list(range(N_CORES)), trace=trace)
    global LAST_RESULTS
    LAST_RESULTS = res
    out = np.asarray(res.results[0]["out"], np.float32).reshape(1, 2)
    return out


# revision 27
# speedup vs baseline: 1.3938x; 1.3938x over previous
"""GCN (3-layer + MLP head) on 8 Trainium2 NeuronCores.

Strategy (graph-parallel, per sharding hint):
  - Nodes sharded 8 ways by id; each core owns its dst-node shard plus the
    edges (excl. self-loops) pointing into it.  Within each shard, nodes are
    permuted into blocks by a greedy 4-vector bin-packing so per-(quarter,
    block) in-edge counts are flat across cores (the shared SPMD schedule
    pays max-over-cores per cell).
  - Phase A (per core): Hs = dinv * (X_shard @ W1) as a bf16 row table kept
    in SBUF + written to DRAM.  The table AllGather is split into 4 block-
    aligned quarter pieces so the first fires as soon as the first 24 blocks
    are done and message passing starts ~110us in.  The per-block
    accumulator is initialized right here to dinv*(XW1) + sqrt(deg) (x) b1
    == self-loop message + pre-scaled bias (so self-loops are never
    gathered and no per-cell bias matmuls exist).
  - MP phase: edges sorted by (src-quarter, dst block) and packed densely
    at cnt_max granularity; dma_gather pulls 256B bf16 rows in 4096-slot
    chunks (short final chunk per quarter); one batched DVE is_equal with
    0-stride broadcast APs builds all one-hot S columns of a chunk; per
    (group x block) incidence a PE matmul scatter-sums messages into the
    dst-block PSUM, accumulated into a [128, nb, F] f32 SBUF accumulator.
  - GCN layers 2+3 collapse to weighted node sums (host norm vectors);
    p = sum_d u2[d]*relu(h1[d]) is a PE reduction, combined across cores by
    a tiny AllGather + ones-vector matmul (the 512B mesh AllReduce costs
    164us; this path ~25us).  Tail MLP replicated; core 0's output is
    returned.
"""
import math
import numpy as np
import ml_dtypes

import concourse.bass as bass
import concourse.tile as tile
from concourse import bacc, mybir
from concourse.bass_utils import run_bass_kernel_spmd

N_CORES = 8
F = 128          # feature dim (all layers)
BLK = 128        # dst-block size (PSUM partition dim)
GRP = 128        # edges per matmul group (PE contraction dim)
CHUNK = 4096     # slots per dma_gather call
CGRPS = CHUNK // GRP
NQUEUES = 4
SENT = 130.0     # dst offset sentinel for padded edge slots (!= 0..127)

BF16 = ml_dtypes.bfloat16


# ----------------------------------------------------------------------------
# host preprocessing: shard, normalize, sort, pack, schedule
# ----------------------------------------------------------------------------
def _preprocess(graph, edge_index, rates, params):
    N = graph.shape[0]
    assert N % N_CORES == 0
    shard = N // N_CORES                      # real nodes per core
    nb = math.ceil(shard / BLK)               # dst blocks per core
    shard_pad = nb * BLK
    nq = 4                                    # src quarters (int16 idx range)

    src = np.asarray(edge_index[0], np.int64)
    dst = np.asarray(edge_index[1], np.int64)

    # degrees / normalization (float64 host precompute of scalar edge data)
    deg = np.bincount(dst, minlength=N).astype(np.float64) + 1.0
    dinv = deg ** -0.5
    sq = deg ** 0.5
    u1 = dinv * (np.bincount(src, weights=dinv[dst], minlength=N) + dinv)
    y = u1 * dinv
    u2 = dinv * (np.bincount(src, weights=y[dst], minlength=N) + y)
    S1 = float(u1.sum())
    S2 = float(u2.sum())

    # quarters = block-aligned slices of each shard (mod-shard interleave)
    # so that AllGather_q covers every core's q-th block range and can be
    # issued as soon as those phase-A blocks are done.
    qnb = [24, 25, 24, 25]                    # blocks per quarter (sum = nb)
    assert sum(qnb) == nb
    qb0 = np.cumsum([0] + qnb)                # block offsets [nq+1]
    qsz = [q * BLK for q in qnb]              # rows per (core, quarter)
    assert max(qsz) * N_CORES - 1 <= np.iinfo(np.int16).max
    blk2q = np.zeros(nb, np.int64)
    for q in range(nq):
        blk2q[qb0[q]:qb0[q + 1]] = q

    # ---- balance per-(quarter, block) in-edge counts by permuting each
    #      shard's node->block assignment (greedy 4-vector bin packing,
    #      2 rounds since src quarters depend on the placement).  The
    #      shared schedule pays max-over-cores per cell, so flattening the
    #      spread cuts gather slots on both the Q7 and SDMA side.
    lpos = np.arange(N) % shard               # local position within shard
    cap = np.full(nb, BLK, np.int64)
    cap[-1] = shard - (nb - 1) * BLK
    for _ in range(3):
        src_q = blk2q[lpos[src] // BLK]
        qdeg = np.zeros((N, nq), np.int64)
        np.add.at(qdeg, (dst, src_q), 1)
        new_lpos = np.empty(N, np.int64)
        for c in range(N_CORES):
            sl = slice(c * shard, (c + 1) * shard)
            qd = qdeg[sl].astype(np.float64)
            order = np.argsort(-qd.sum(1), kind="stable")
            loads = np.zeros((nb, nq))
            fill = np.zeros(nb, np.int64)
            pos = np.empty(shard, np.int64)
            for i in order:
                sc = (loads + qd[i]).max(1)
                sc[fill >= cap] = np.inf
                b = int(np.argmin(sc))
                loads[b] += qd[i]
                pos[i] = b * BLK + fill[b]
                fill[b] += 1
            new_lpos[sl] = pos
        lpos = new_lpos

    # per-core edge attribution (NO self loops in the gather)
    core_of = dst // shard
    l_src = lpos[src]                         # local row of src in its shard
    sblk = l_src // BLK
    quarter = blk2q[sblk]
    c_src = src // shard
    qsz_a = np.array(qsz, np.int64)
    qoff_a = qb0[:-1] * BLK
    loc_idx = c_src * qsz_a[quarter] + (l_src - qoff_a[quarter])
    dl = lpos[dst]
    blk_of = dl // BLK
    off_of = (dl % BLK).astype(np.int64)

    # counts per (core, quarter, block); shared schedule uses max over cores
    cnt = np.zeros((N_CORES, nq, nb), np.int64)
    np.add.at(cnt, (core_of, quarter, blk_of), 1)
    cnt_max = cnt.max(axis=0)                       # [nq, nb]

    # dense cell offsets within each quarter's slot space
    cell_off = np.zeros((nq, nb + 1), np.int64)
    for q in range(nq):
        cell_off[q, 1:] = np.cumsum(cnt_max[q])
    SQ = cell_off[:, -1]                            # slots per quarter

    # shared chunk-level schedule
    sched_chunks = []   # dicts: q, k, nreg, entries=[(j, gl, b, first, last)]
    chunk_index = {}    # (q, k) -> global chunk idx
    for q in range(nq):
        nck = math.ceil(int(SQ[q]) / CHUNK)
        per_chunk = [[] for _ in range(nck)]
        for b in range(nb):
            o, e = int(cell_off[q, b]), int(cell_off[q, b + 1])
            g0, g1 = o // GRP, (e - 1) // GRP
            for g in range(g0, g1 + 1):
                ch = g // CGRPS
                per_chunk[ch].append((g - ch * CGRPS, b, g == g0, g == g1))
        for k in range(nck):
            entries = [(j, gl, b, fi, la)
                       for j, (gl, b, fi, la) in enumerate(per_chunk[k])]
            nreg = min(CHUNK, int(SQ[q]) - k * CHUNK)
            chunk_index[(q, k)] = len(sched_chunks)
            sched_chunks.append(dict(q=q, k=k, nreg=nreg, entries=entries))
    nchunk = len(sched_chunks)
    ncol = max(len(c["entries"]) for c in sched_chunks)

    # per-core edge data arrays in schedule order
    idx16 = np.zeros((N_CORES, nchunk, 128, CHUNK // 16), np.int16)
    dstid = np.full((N_CORES, nchunk, 128, ncol), SENT, BF16)
    e_ar = np.arange(CHUNK)
    for c in range(N_CORES):
        m = core_of == c
        q_c, b_c = quarter[m], blk_of[m]
        il_c, off_c = loc_idx[m], off_of[m]
        order = np.lexsort((b_c, q_c))
        q_c, b_c, il_c, off_c = q_c[order], b_c[order], il_c[order], off_c[order]
        key = q_c * nb + b_c
        for q in range(nq):
            nck = math.ceil(int(SQ[q]) / CHUNK)
            SQp = nck * CHUNK
            iv = np.zeros(SQp, np.int64)
            ov = np.full(SQp, SENT)
            cell = np.full(SQp, -1, np.int64)
            for b in range(nb):
                cc = int(cnt[c, q, b])
                o = int(cell_off[q, b])
                s0 = np.searchsorted(key, q * nb + b, 'left')
                iv[o:o + cc] = il_c[s0:s0 + cc]
                ov[o:o + cc] = off_c[s0:s0 + cc]
                cell[o:o + cc] = b
            for k in range(nck):
                ci = chunk_index[(q, k)]
                sl = slice(k * CHUNK, (k + 1) * CHUNK)
                tmp = np.zeros((16, CHUNK // 16), np.int16)
                tmp[e_ar % 16, e_ar // 16] = iv[sl]
                idx16[c, ci] = np.tile(tmp, (8, 1))
                for (j, gl, b, fi, la) in sched_chunks[ci]["entries"]:
                    s = k * CHUNK + gl * GRP
                    seg = slice(s, s + GRP)
                    dstid[c, ci][:, j] = np.where(
                        cell[seg] == b, ov[seg], SENT).astype(BF16)

    # phase A inputs
    X = np.asarray(graph, np.float32)
    xt = np.zeros((N_CORES, F, shard_pad), BF16)
    dinv_pm = np.zeros((N_CORES, BLK, nb), np.float32)
    sq_pm = np.zeros((N_CORES, BLK, nb), np.float32)
    u2_pm = np.zeros((N_CORES, BLK, nb), np.float32)
    for c in range(N_CORES):
        lp = lpos[c * shard: (c + 1) * shard]
        p2n = np.empty(shard, np.int64)
        p2n[lp] = np.arange(shard)            # position -> local node idx
        xs = X[c * shard: (c + 1) * shard][p2n]
        xt[c, :, :shard] = xs.T.astype(BF16)
        dv = np.zeros(shard_pad); dv[:shard] = dinv[c * shard:(c + 1) * shard][p2n]
        sv = np.zeros(shard_pad); sv[:shard] = sq[c * shard:(c + 1) * shard][p2n]
        uv = np.zeros(shard_pad); uv[:shard] = u2[c * shard:(c + 1) * shard][p2n]
        dinv_pm[c] = dv.reshape(nb, BLK).T
        sq_pm[c] = sv.reshape(nb, BLK).T
        u2_pm[c] = uv.reshape(nb, BLK).T

    p = params
    col = lambda v: np.asarray(v, np.float32).reshape(-1, 1)
    iota_bf = np.tile(np.arange(BLK, dtype=np.float32)[None, :],
                      (128, 1)).astype(BF16)
    common = dict(
        w1=np.asarray(p["conv1_W"], np.float32).astype(BF16),
        b1t=np.tile(np.asarray(p["conv1_b"], np.float32).reshape(1, F),
                    (BLK, 1)),
        iotabf=iota_bf,
        rates_col=col(rates),
        encw1=np.asarray(p["enc_W1"], np.float32),
        encb1=col(p["enc_b1"]),
        encw2=np.asarray(p["enc_W2"], np.float32),
        encb2=col(p["enc_b2"]),
        w2a=np.asarray(p["conv2_W"], np.float32)[:F],
        w2b=np.asarray(p["conv2_W"], np.float32)[F:],
        b2col=col(p["conv2_b"]),
        s1col=np.full((F, 1), S1, np.float32),
        s2col=np.full((F, 1), S2, np.float32),
        w3=np.asarray(p["conv3_W"], np.float32),
        b3col=col(p["conv3_b"]),
        hidw=np.asarray(p["hid_W"], np.float32),
        hidb=np.asarray(p["hid_b"], np.float32).reshape(2, F).T,
        hid2wa=np.asarray(p["hid2_W"], np.float32)[:F],
        hid2wb=np.asarray(p["hid2_W"], np.float32)[F:],
        hid2b=col(p["hid2_b"]),
        finw=np.asarray(p["fin_W"], np.float32),
        finb=col(p["fin_b"]),
    )
    in_maps = []
    for c in range(N_CORES):
        m = dict(common)
        m.update(
            xt=xt[c], dinv=dinv_pm[c], sqdeg=sq_pm[c], u2c=u2_pm[c],
            idx16=idx16[c], dstid=dstid[c],
        )
        in_maps.append(m)

    dims = dict(N=N, shard=shard, shard_pad=shard_pad, nb=nb, nq=nq,
                qnb=tuple(qnb), nchunk=nchunk, ncol=ncol)
    return in_maps, sched_chunks, dims


# ----------------------------------------------------------------------------
# device program
# ----------------------------------------------------------------------------
def _build(sched_chunks, dims):
    nb, nq, nchunk = dims["nb"], dims["nq"], dims["nchunk"]
    ncol = dims["ncol"]
    shard_pad, N = dims["shard_pad"], dims["N"]
    qnb = list(dims["qnb"])
    qb0 = [0]
    for q in qnb:
        qb0.append(qb0[-1] + q)
    f32, bf16, i16 = mybir.dt.float32, mybir.dt.bfloat16, mybir.dt.int16

    nc = bacc.Bacc("TRN2", target_bir_lowering=False, debug=False,
                   num_devices=N_CORES, num_swdge_queues=NQUEUES)
    I = lambda name, shape, dt=f32: nc.dram_tensor(name, shape, dt, kind="ExternalInput")
    xt_e = I("xt", [F, shard_pad], bf16)
    w1_e = I("w1", [F, F], bf16)
    b1t_e = I("b1t", [BLK, F])
    dinv_e = I("dinv", [BLK, nb]); sq_e = I("sqdeg", [BLK, nb])
    u2_e = I("u2c", [BLK, nb])
    idx_e = I("idx16", [nchunk, 128, CHUNK // 16], i16)
    dst_e = I("dstid", [nchunk, 128, ncol], bf16)
    iota_e = I("iotabf", [128, BLK], bf16)
    rates_e = I("rates_col", [16, 1])
    encw1_e = I("encw1", [16, 8]); encb1_e = I("encb1", [8, 1])
    encw2_e = I("encw2", [8, F]); encb2_e = I("encb2", [F, 1])
    w2a_e = I("w2a", [F, F]); w2b_e = I("w2b", [F, F]); b2_e = I("b2col", [F, 1])
    s1_e = I("s1col", [F, 1]); s2_e = I("s2col", [F, 1])
    w3_e = I("w3", [F, F]); b3_e = I("b3col", [F, 1])
    hidw_e = I("hidw", [F, 2 * F]); hidb_e = I("hidb", [F, 2])
    hid2wa_e = I("hid2wa", [F, F]); hid2wb_e = I("hid2wb", [F, F])
    hid2b_e = I("hid2b", [F, 1])
    finw_e = I("finw", [F, 2]); finb_e = I("finb", [2, 1])
    out_e = nc.dram_tensor("out", [2, 1], f32, kind="ExternalOutput")

    hs_shard_q = [nc.dram_tensor(f"hs_shard{q}", [qnb[q] * BLK, F], bf16)
                  for q in range(nq)]
    hs_full_q = [nc.dram_tensor(f"hs_full{q}", [N_CORES * qnb[q] * BLK, F],
                                bf16, addr_space="Shared")
                 for q in range(nq)]
    p_dram = nc.dram_tensor("p_dram", [1, F], f32)
    p_all = nc.dram_tensor("p_all", [N_CORES, F], f32, addr_space="Shared")
    groups_all = list(range(N_CORES))

    with tile.TileContext(nc) as tc:
        with (
            tc.tile_pool(name="const", bufs=1) as cpool,
            tc.tile_pool(name="xa", bufs=3) as xapool,
            tc.tile_pool(name="work", bufs=8) as wpool,
            tc.tile_pool(name="gat", bufs=5) as gpool,
            tc.tile_pool(name="sstile", bufs=3) as spool,
            tc.tile_pool(name="accum", bufs=1) as apool,
            tc.tile_pool(name="ps", bufs=4, space="PSUM") as pspool,
            tc.tile_pool(name="psp", bufs=1, space="PSUM") as psppool,
        ):
            # ---- constants / small inputs
            w1_sb = cpool.tile([F, F], bf16); nc.sync.dma_start(w1_sb[:], w1_e[:])
            b1t_sb = cpool.tile([BLK, F], f32); nc.sync.dma_start(b1t_sb[:], b1t_e[:])
            dinv_sb = cpool.tile([BLK, nb], f32); nc.sync.dma_start(dinv_sb[:], dinv_e[:])
            sq_sb = cpool.tile([BLK, nb], f32); nc.sync.dma_start(sq_sb[:], sq_e[:])
            u2_sb = cpool.tile([BLK, nb], f32); nc.sync.dma_start(u2_sb[:], u2_e[:])
            iota_sb = cpool.tile([128, BLK], bf16); nc.sync.dma_start(iota_sb[:], iota_e[:])

            hs_big = cpool.tile([BLK, nb, F], bf16)
            accum = apool.tile([BLK, nb, F], f32)

            # ---- phase A: table Hs = dinv*(X@W1) + accum init
            #      accum[:,n,:] = dinv*(XW1) + sqrt(deg) (x) b1
            #      (= self-loop message + pre-scaled bias)
            #      xt loads batched 4 blocks; hs stores batched per quarter
            #      chunklet; AllGather_q fires as soon as quarter q is done.
            for n4 in range(0, nb, 4):
                hi = min(n4 + 4, nb)
                w = hi - n4
                xts = xapool.tile([F, 4 * BLK], bf16, tag="xts")
                nc.sync.dma_start(xts[:, :w * BLK],
                                  xt_e[:, n4 * BLK: hi * BLK])
                for n in range(n4, hi):
                    s = (n - n4) * BLK
                    psA = pspool.tile([BLK, F], f32, tag="ps")
                    nc.tensor.matmul(psA[:], xts[:, s: s + BLK], w1_sb[:],
                                     start=True, stop=True)
                    nc.scalar.activation(
                        hs_big[:, n, :], psA[:],
                        mybir.ActivationFunctionType.Copy,
                        scale=dinv_sb[:, n: n + 1],
                    )
                    nc.vector.tensor_copy(accum[:, n, :], hs_big[:, n, :])
                    nc.vector.scalar_tensor_tensor(
                        out=accum[:, n, :], in0=b1t_sb[:],
                        scalar=sq_sb[:, n: n + 1], in1=accum[:, n, :],
                        op0=mybir.AluOpType.mult, op1=mybir.AluOpType.add,
                    )
            # quarter-sliced table stores + per-quarter AllGathers
            for q in range(nq):
                for b0 in range(qb0[q], qb0[q + 1], 4):
                    b1_ = min(b0 + 4, qb0[q + 1])
                    r0 = (b0 - qb0[q]) * BLK
                    dst_ap = hs_shard_q[q][r0: r0 + (b1_ - b0) * BLK, :]
                    dst_ap = dst_ap.rearrange("(j p) f -> p j f", p=BLK)
                    nc.sync.dma_start(dst_ap, hs_big[:, b0: b1_, :])
                nc.gpsimd.collective_compute(
                    "AllGather", mybir.AluOpType.bypass,
                    replica_groups=[groups_all],
                    ins=[hs_shard_q[q][:]], outs=[hs_full_q[q][:]],
                )

            # ---- message passing over chunks
            ps_p = psppool.tile([1, F], f32)
            p_started = False
            psum_of_block = {}
            for ci, chd in enumerate(sched_chunks):
                q = chd["q"]
                # short final chunks: gather only the slots the schedule uses
                ng = min(CGRPS, -(-chd["nreg"] // GRP))
                ni = ng * GRP
                idxt = wpool.tile([128, CHUNK // 16], i16, tag="idxt")
                nc.sync.dma_start(idxt[:, : ni // 16], idx_e[ci][:, : ni // 16])
                dstt = wpool.tile([128, ncol], bf16, tag="dstt")
                nc.sync.dma_start(dstt[:], dst_e[ci])
                G = gpool.tile([128, CGRPS, F], bf16, tag="G")
                nc.gpsimd.dma_gather(
                    out_ap=G[:, :ng, :],
                    in_ap=hs_full_q[q][:],
                    idxs_ap=idxt[:, : ni // 16],
                    num_idxs=ni, num_idxs_reg=ni, elem_size=F,
                    single_packet=False, queue_num=(ci % NQUEUES),
                )
                S_all = spool.tile([128, ncol, BLK], bf16, tag="S")
                nc.vector.tensor_tensor(
                    out=S_all[:],
                    in0=dstt[:].unsqueeze(2).to_broadcast([128, ncol, BLK]),
                    in1=iota_sb[:].unsqueeze(1).to_broadcast([128, ncol, BLK]),
                    op=mybir.AluOpType.is_equal,
                )
                for (j, gl, b, first, last) in chd["entries"]:
                    if first:
                        psb = pspool.tile([BLK, F], f32, tag="ps")
                        psum_of_block[b] = psb
                    psb = psum_of_block[b]
                    nc.tensor.matmul(
                        psb[:], S_all[:, j, :], G[:, gl, :],
                        start=first, stop=last,
                    )
                    if last:
                        del psum_of_block[b]
                        if q < nq - 1:
                            nc.vector.tensor_add(accum[:, b, :], accum[:, b, :], psb[:])
                        else:
                            nc.vector.tensor_add(accum[:, b, :], accum[:, b, :], psb[:])
                            h1b = spool.tile([BLK, F], f32, tag="h1b")
                            nc.scalar.activation(
                                h1b[:], accum[:, b, :],
                                mybir.ActivationFunctionType.Relu,
                                scale=dinv_sb[:, b: b + 1],
                            )
                            nc.tensor.matmul(
                                ps_p[:], u2_sb[:, b: b + 1], h1b[:],
                                start=not p_started, stop=(b == nb - 1),
                                skip_group_check=True,
                            )
                            p_started = True

            # ---- combine p across cores: tiny AllGather + ones reduction
            p_sb = cpool.tile([1, F], f32)
            nc.vector.tensor_copy(p_sb[:], ps_p[:])
            nc.sync.dma_start(p_dram[:], p_sb[:])
            nc.gpsimd.collective_compute(
                "AllGather", mybir.AluOpType.bypass,
                replica_groups=[groups_all],
                ins=[p_dram[:]], outs=[p_all[:]],
            )
            p_all_sb = cpool.tile([N_CORES, F], f32)
            nc.sync.dma_start(p_all_sb[:], p_all[:])
            ones8 = cpool.tile([N_CORES, 1], f32)
            nc.vector.memset(ones8[:], 1.0)
            ps_pr = pspool.tile([1, F], f32, tag="ps")
            nc.tensor.matmul(ps_pr[:], ones8[:], p_all_sb[:], start=True, stop=True)
            p_row = cpool.tile([1, F], f32)
            nc.vector.tensor_copy(p_row[:], ps_pr[:])
            id1 = cpool.tile([1, 1], f32)
            nc.vector.memset(id1[:], 1.0)
            psT = pspool.tile([F, 1], f32, tag="ps")
            nc.tensor.transpose(psT[:], p_row[:], id1[:])
            p_col = cpool.tile([F, 1], f32)
            nc.vector.tensor_copy(p_col[:], psT[:])

            # ---- replicated tail MLP (column-vector chain on PE/ACT/DVE)
            tl = cpool
            def ld(e, shape, dt=f32):
                t = tl.tile(shape, dt, tag=f"c_{e.name}")
                nc.sync.dma_start(t[:], e[:])
                return t
            rates_sb = ld(rates_e, [16, 1]); encw1_sb = ld(encw1_e, [16, 8])
            encb1_sb = ld(encb1_e, [8, 1]); encw2_sb = ld(encw2_e, [8, F])
            encb2_sb = ld(encb2_e, [F, 1])
            w2a_sb = ld(w2a_e, [F, F]); w2b_sb = ld(w2b_e, [F, F])
            b2_sb = ld(b2_e, [F, 1]); s1_sb = ld(s1_e, [F, 1]); s2_sb = ld(s2_e, [F, 1])
            w3_sb = ld(w3_e, [F, F]); b3_sb = ld(b3_e, [F, 1])
            hidw_sb = ld(hidw_e, [F, 2 * F]); hidb_sb = ld(hidb_e, [F, 2])
            hid2wa_sb = ld(hid2wa_e, [F, F]); hid2wb_sb = ld(hid2wb_e, [F, F])
            hid2b_sb = ld(hid2b_e, [F, 1])
            finw_sb = ld(finw_e, [F, 2]); finb_sb = ld(finb_e, [2, 1])

            pst = pspool.tile([F, 2], f32, tag="ps")  # scratch psum, 2 cols

            # r1 = relu(encW1^T rates + encb1)   [8,1]
            nc.tensor.matmul(pst[:8, 0:1], encw1_sb[:], rates_sb[:], start=True, stop=True)
            r1 = tl.tile([8, 1], f32)
            nc.scalar.activation(r1[:], pst[:8, 0:1],
                                 mybir.ActivationFunctionType.Relu, bias=encb1_sb[:])
            # r2 = encW2^T r1 + encb2            [F,1]
            nc.tensor.matmul(pst[:, 1:2], encw2_sb[:], r1[:], start=True, stop=True)
            r2 = tl.tile([F, 1], f32)
            nc.vector.tensor_add(r2[:], pst[:, 1:2], encb2_sb[:])
            # m_r = S2 * r2
            mr = tl.tile([F, 1], f32)
            nc.vector.tensor_mul(mr[:], r2[:], s2_sb[:])
            # u1h2 = W2a^T p + W2b^T m_r + S1*b2 ; q = u1h2 / N
            pst2 = pspool.tile([F, 1], f32, tag="ps")
            nc.tensor.matmul(pst2[:], w2a_sb[:], p_col[:], start=True, stop=False)
            nc.tensor.matmul(pst2[:], w2b_sb[:], mr[:], start=False, stop=True)
            sb2 = tl.tile([F, 1], f32)
            nc.vector.tensor_mul(sb2[:], b2_sb[:], s1_sb[:])
            qv = tl.tile([F, 1], f32)
            nc.vector.tensor_add(qv[:], pst2[:], sb2[:])
            nc.vector.tensor_scalar_mul(qv[:], qv[:], 1.0 / dims["N"])
            # m3 = W3^T q + b3
            pst3 = pspool.tile([F, 1], f32, tag="ps")
            nc.tensor.matmul(pst3[:], w3_sb[:], qv[:], start=True, stop=True)
            m3 = tl.tile([F, 1], f32)
            nc.vector.tensor_add(m3[:], pst3[:], b3_sb[:])
            # g1 = relu(hidW^T m3 + hidb)  [256] as two cols
            g1a = tl.tile([F, 1], f32); g1b = tl.tile([F, 1], f32)
            nc.tensor.matmul(pst[:, 0:1], hidw_sb[:, :F], m3[:], start=True, stop=True)
            nc.scalar.activation(g1a[:], pst[:, 0:1],
                                 mybir.ActivationFunctionType.Relu, bias=hidb_sb[:, 0:1])
            nc.tensor.matmul(pst[:, 1:2], hidw_sb[:, F:], m3[:], start=True, stop=True)
            nc.scalar.activation(g1b[:], pst[:, 1:2],
                                 mybir.ActivationFunctionType.Relu, bias=hidb_sb[:, 1:2])
            # g2 = relu(hid2W^T g1 + hid2b)  [F,1]
            pst4 = pspool.tile([F, 1], f32, tag="ps")
            nc.tensor.matmul(pst4[:], hid2wa_sb[:], g1a[:], start=True, stop=False)
            nc.tensor.matmul(pst4[:], hid2wb_sb[:], g1b[:], start=False, stop=True)
            g2 = tl.tile([F, 1], f32)
            nc.scalar.activation(g2[:], pst4[:],
                                 mybir.ActivationFunctionType.Relu, bias=hid2b_sb[:])
            # out = finW^T g2 + finb  [2,1]
            pst5 = pspool.tile([2, 1], f32, tag="ps")
            nc.tensor.matmul(pst5[:], finw_sb[:], g2[:], start=True, stop=True)
            outv = tl.tile([2, 1], f32)
            nc.vector.tensor_add(outv[:], pst5[:], finb_sb[:])
            nc.sync.dma_start(out_e[:], outv[:])

    nc.compile()
    return nc


_CACHE = {}
LAST_RESULTS = None


def kernel(**inputs):
    graph = np.asarray(inputs["graph"], np.float32)
    edge_index = np.asarray(inputs["edge_index"], np.int64)
    rates = np.asarray(inputs["rates"], np.float32)
    params = {k: np.asarray(v) for k, v in inputs.items()
              if k not in ("graph", "edge_index", "rates")}
    in_maps, sched, dims = _preprocess(graph, edge_index, rates, params)
    key = (dims["nchunk"], dims["ncol"], dims["shard_pad"],
           tuple((c["q"], c["nreg"], tuple(c["entries"])) for c in sched))
    if key not in _CACHE:
        _CACHE[key] = _build(sched, dims)
    nc = _CACHE[key]
    import os
    trace = bool(int(os.environ.get("GCN_TRACE", "0")))
    res = run_bass_kernel_spmd(nc, in_maps, list(range(N_CORES)), trace=trace)
    global LAST_RESULTS
    LAST_RESULTS = res
    out = np.asarray(res.results[0]["out"], np.float32).reshape(1, 2)
    return out
